# revision 41
# baseline (speedup 1.0000x reference)
"""Trainium2 Bass kernel for nn_Encoder_Spatio (gnn_message_passing).

Math (exact reformulation of the reference):
    h1 = ELU(x @ fc1_w.T + b1)
    h2 = ELU(h1 @ fc2_w.T + b2)
    probs[b,i,j] = sq[b,i] + sk[b,j] + c
where BN (eval) and the Q/K projections + mlp2 halves fold on the host into
    vq, vk in R^256,  c scalar:    sq = h2 @ vq,  sk = h2 @ vk  (+consts).

ELU identity used on device:
    ELU(z)+1 = min(exp(z+b), max(z+b+1, 1))
with the -1 folded into the next consumer's bias on the host. Two engine
splits of that identity are used ("plan" per token slice):
    A (slices 0-1):  e  = Exp(z+b)            ACT, PSUM -> bf16
                     r  = max(z+(b+1), 1)     DVE TS, PSUM -> bf16 (1x)
                     h  = min(e, r)           DVE TT bf16 (2x)
    D (slices 2-3):  e  = Exp(z+b)            ACT, PSUM -> bf16
                     u  = z+(b+1)             ACT Identity, PSUM -> bf16
                     r  = max(u, 1)           DVE TS bf16 all-SBUF (4x)
                     h  = min(r, e)           DVE TT bf16 (2x)
Plan D moves the second PSUM read (inherently 1x) onto ACT so DVE keeps only
cheap 2x/4x bf16 ops; A is used early (when few emissions load DVE), D late.
Both engines run ~95% busy through the mid-phase - the schedule is at the
two-engine work balance point, which is the real wall for this kernel.

The output is written to HBM in bf16 and upcast to fp32 on the host: the
correctness gate is 2e-2 relative error, the bf16 compute path measures
~5.2e-3, and bf16 rounding of the scores adds ~1.7e-3 in quadrature. This
halves the dominant HBM write stream (16.8 MB -> 8.4 MB per core) and makes
the emission adds eligible for the DVE 4x perf mode (all-SBUF, 2-byte
packed: ~0.35us per [128,512] vs ~0.70us fp32), which is what lets the
element production keep up with the halved DMA stream.

Score emission: out[i,j] = t0[j] + sq[i] with t0 = sk + c kept in bf16.
Units of [128, 4 row blocks, 512 cols] (0.5 MB, one tensor_scalar per row
block, all on DVE mid-phase), plus the last slice: column-strip units with
a DVE/ACT mix (ACT is idle once the MLP drains) and the final rows as
full-width [128, 2, 2048] units (4 KB descriptors). All output DMAs issue
from the otherwise-idle sync ring. The pipeline is software-pipelined with
one slice of lookahead: A(s+1)'s matmuls are hoisted ahead of B(s) (the PE
queue is in-order), its ELU runs right after B(s)'s, and each stage's score
matmuls/copies precede its emission burst.

Sharding: pure data-parallel over B (8 batches -> 8 cores), no collectives.
"""

import sys

if "/opt/trn_rl_repo" not in sys.path:
    sys.path.insert(0, "/opt/trn_rl_repo")

import types

import numpy as np


def _ensure_axon_hooks():
    """concourse.bass_utils imports antenv.axon_hooks when tracing is
    requested; this image's antenv package lacks that submodule, which turns
    a skipped-trace fallback into a hard ImportError. Fill the hole with a
    None-hook stub (tracing degrades gracefully) if it's missing."""
    try:
        import antenv.axon_hooks  # noqa: F401
        return
    except ImportError:
        pass
    try:
        import antenv
    except ImportError:
        return
    mod = types.ModuleType("antenv.axon_hooks")
    mod._hook = None

    def set_axon_ntff_profile_hook(hook):
        mod._hook = hook

    def get_axon_ntff_profile_hook():
        return mod._hook

    mod.set_axon_ntff_profile_hook = set_axon_ntff_profile_hook
    mod.get_axon_ntff_profile_hook = get_axon_ntff_profile_hook
    sys.modules["antenv.axon_hooks"] = mod
    antenv.axon_hooks = mod


_ensure_axon_hooks()

import ml_dtypes

from concourse import bass, tile, mybir
from concourse.bass_utils import run_bass_kernel_spmd

B, N, F = 8, 2048, 256      # batch, tokens, feature width (NIN == NHID == 256)
KC = F // 128               # feature chunks of 128 partitions
TS = 4                      # token slices
SW = N // TS                # slice width (512 = PSUM bank / max matmul moving dim)
RBW = 128                   # output row-block width (partition dim)
RB_PER_TS = SW // RBW       # row blocks per token slice (= sq group size)

F32 = mybir.dt.float32
BF16 = mybir.dt.bfloat16
AF = mybir.ActivationFunctionType
ALU = mybir.AluOpType

# bf16 weight blobs: wb1 = w1 chunks (needed before the first matmul),
# wb2 = w2 chunks + vk-replicated + vq (needed one stage later)
W1_COLS = KC * F
W2_OFF = 0                      # within wb2
VKB_OFF = W2_OFF + KC * F       # KC chunks of [128, 128] (vk replicated)
VQ_OFF = VKB_OFF + KC * 128     # [128, 2*KC] (vq at even cols, zero odd)
WB2_COLS = VQ_OFF + 2 * KC

WARMUP_N = 10               # dummy matmuls for the PE p-state ramp
WARMUP_MOVING = 192
PLAN = {0: "A", 1: "A", 2: "D", 3: "D"}   # ELU engine split per token slice


def _split_multiwaits(nc):
    """This walrus build lowers at most one sync-wait per instruction on some
    instruction classes (the TileContext exit drain trips it). Hoist extra
    waits onto preceding single-wait drains on the same engine."""
    for f in nc.m.functions:
        for bb in f.blocks:
            insts = list(bb.instructions)
            out = []
            changed = False
            for inst in insts:
                si = inst.sync_info
                if si is not None and si.on_wait and len(si.on_wait) > 1:
                    waits = list(si.on_wait)
                    for k, w in enumerate(waits[:-1]):
                        d = mybir.InstDrain(name=f"{inst.name}-ws{k}")
                        d.engine = inst.engine
                        d.sync_info = mybir.SyncInfo(on_wait=[w], on_update=[])
                        out.append(d)
                    inst.sync_info = mybir.SyncInfo(
                        on_wait=[waits[-1]], on_update=list(si.on_update)
                    )
                    changed = True
                out.append(inst)
            if changed:
                bb.instructions = out


def _build_program():
    nc = bass.Bass(trn_type="TRN2")

    xt_d = nc.dram_tensor("xt", [F, N], BF16, kind="ExternalInput")
    wb1_d = nc.dram_tensor("wb1", [128, W1_COLS], BF16, kind="ExternalInput")
    wb2_d = nc.dram_tensor("wb2", [128, WB2_COLS], BF16, kind="ExternalInput")
    cst_d = nc.dram_tensor("cst", [128, 4 * KC + 1], F32, kind="ExternalInput")
    out_d = nc.dram_tensor("out", [N, N], BF16, kind="ExternalOutput")

    with tile.TileContext(nc) as tc:
        with (
            tc.tile_pool(name="wts", bufs=1) as wpool,
            tc.tile_pool(name="eh", bufs=4) as epool,
            tc.tile_pool(name="uh", bufs=4) as upool,
            tc.tile_pool(name="rh", bufs=4) as rpool,
            tc.tile_pool(name="h1", bufs=2) as h1pool,
            tc.tile_pool(name="h2", bufs=2) as h2pool,
            tc.tile_pool(name="t0", bufs=1) as t0pool,
            tc.tile_pool(name="sqs", bufs=TS) as sqpool,
            tc.tile_pool(name="ob", bufs=10) as opool,
            tc.tile_pool(name="obr", bufs=3) as orow_pool,
            tc.tile_pool(name="psmm", bufs=6, space="PSUM") as pspool,
            tc.tile_pool(name="pst0", bufs=1, space="PSUM") as t0ps_pool,
            tc.tile_pool(name="pssq", bufs=1, space="PSUM") as sqps_pool,
        ):
            # ---- weights / constants (live for the whole kernel) ----
            wb1 = wpool.tile([128, W1_COLS], BF16, tag="wb1", name="wb1")
            wb2 = wpool.tile([128, WB2_COLS], BF16, tag="wb2", name="wb2")
            cst = wpool.tile([128, 4 * KC + 1], F32, tag="cst", name="cst")
            x0 = wpool.tile([128, KC, SW], BF16, tag="x0", name="x0")
            x1 = wpool.tile([128, KC, SW], BF16, tag="x1", name="x1")
            x23 = wpool.tile([128, KC, 2 * SW], BF16, tag="x23", name="x23")

            def w1ap(k, oc):
                return wb1[:, k * F + oc * 128:k * F + oc * 128 + 128]

            def w2ap(k, oc):
                return wb2[:, W2_OFF + k * F + oc * 128:W2_OFF + k * F + oc * 128 + 128]

            def vkbap(k):
                return wb2[:, VKB_OFF + k * 128:VKB_OFF + (k + 1) * 128]

            vqap = wb2[:, VQ_OFF:VQ_OFF + 2 * KC]
            b1e = cst[:, 0:KC]
            b1r = cst[:, KC:2 * KC]
            b2e = cst[:, 2 * KC:3 * KC]
            b2r = cst[:, 3 * KC:4 * KC]
            cb = cst[:, 4 * KC:4 * KC + 1]

            def xap(s, k):
                if s == 0:
                    return x0[:, k, :]
                if s == 1:
                    return x1[:, k, :]
                return x23[:, k, (s - 2) * SW:(s - 1) * SW]

            # input on the scalar HWDGE ring, one DMA per pipeline entry
            # point so A(s) only waits on its own slice's completion sem;
            # weights on the sync ring, wb1 first so the first matmul's
            # inputs land first.
            xt_r = xt_d[:].rearrange("(k p) c -> p k c", k=KC)
            dact = wpool.tile([128, 1], F32, tag="dact", name="dact")
            nc.gpsimd.memset(dact[:], 0.0)
            nc.scalar.dma_start(x0[:], xt_r[:, :, 0:SW])
            # dummy Exp right after the x0 trigger: its function-table load
            # (~1.3us of ACT) finishes ~2us before the first real Exp needs
            # it, instead of just barely in time.
            nc.scalar.activation(dact[:], dact[:], AF.Exp)
            nc.scalar.dma_start(x1[:], xt_r[:, :, SW:2 * SW])
            nc.scalar.dma_start(x23[:], xt_r[:, :, 2 * SW:N])
            nc.sync.dma_start(wb1[:], wb1_d[:])
            nc.sync.dma_start(cst[:], cst_d[:])
            nc.sync.dma_start(wb2[:], wb2_d[:])

            # Warmups during the load window: dummy matmuls spin up the PE
            # p-state clock.
            dmw = wpool.tile([128, 128], BF16, tag="dmw", name="dmw")
            dmx = wpool.tile([128, WARMUP_MOVING], BF16, tag="dmx", name="dmx")
            nc.gpsimd.memset(dmw[:], 0.0)
            nc.gpsimd.memset(dmx[:], 0.0)
            dps = pspool.tile([128, SW], F32, name="dps", tag="psmm")
            for w_i in range(WARMUP_N):
                nc.tensor.matmul(dps[:, 0:WARMUP_MOVING], dmw[:], dmx[:],
                                 start=(w_i == 0), stop=(w_i == WARMUP_N - 1))
            # second dummy Exp after the warmups: keeps the ACT queue's
            # pacing ahead of the first real Exp (measured ~1.4us better
            # than without it).
            nc.scalar.activation(dact[:], dact[:], AF.Exp)

            t0_full = t0pool.tile([128, N], BF16, name="t0_full", tag="t0_full")
            sqt = {}       # slice -> sq tile [128, 2*RB_PER_TS] (even cols)
            h1s = {}       # slice -> [128, KC*SW] bf16 tile
            h2s = {}
            n_emit = 0

            def sq_ap(i):
                """Per-partition sq scalar AP for global row block i."""
                return sqt[i // RB_PER_TS][:, 2 * (i % RB_PER_TS):2 * (i % RB_PER_TS) + 1]

            def mlp_mm(wap, rhs):
                """Matmul phase of one layer: KC PSUM tiles, one per output
                chunk. Issued early so the in-order PE queue stays fed."""
                pss = []
                for oc in range(KC):
                    ps = pspool.tile([128, SW], F32, name="ps_mm", tag="psmm")
                    for k in range(KC):
                        nc.tensor.matmul(
                            ps[:], wap(k, oc), rhs(k),
                            start=(k == 0), stop=(k == KC - 1),
                        )
                    pss.append(ps)
                return pss

            def mlp_elu_chunk(h, ps, be, br, oc, plan):
                """ELU+1 for one output chunk, separate from the matmuls so
                a hoisted stage's ELU never sits ahead of the current
                stage's ELU in the ACT/DVE queues."""
                e = epool.tile([128, SW], BF16, name="e_t")
                nc.scalar.activation(e[:], ps[:], AF.Exp, bias=be[:, oc:oc + 1])
                r = rpool.tile([128, SW], BF16, name="r_t")
                if plan == "A":
                    nc.vector.tensor_scalar(
                        r[:], ps[:], br[:, oc:oc + 1], 1.0, ALU.add, ALU.max
                    )
                else:
                    u = upool.tile([128, SW], BF16, name="u_t")
                    nc.scalar.activation(
                        u[:], ps[:], AF.Identity, bias=br[:, oc:oc + 1]
                    )
                    nc.vector.tensor_scalar(r[:], u[:], 1.0, None, ALU.max)
                nc.vector.tensor_tensor(
                    h[:, oc * SW:(oc + 1) * SW], e[:], r[:], ALU.min
                )

            def mlp_elu(h, pss, be, br, plan):
                for oc in range(KC):
                    mlp_elu_chunk(h, pss[oc], be, br, oc, plan)

            def emit_op(dst, src, i, eng):
                """One row block's score add: dst = src + sq[i] (bf16)."""
                if eng == "S":
                    nc.scalar.activation(dst, src, AF.Identity, bias=sq_ap(i))
                else:
                    nc.vector.tensor_scalar(dst, src, sq_ap(i), None, ALU.add)

            def emit_unit(g, s, engs="VVVV"):
                """Ship sq-group g (row blocks 4g..4g+3) at column slice s:
                four [128, SW] adds into one [128, 4, SW] bf16 tile + one
                0.5 MB DMA from the sync ring."""
                ot = opool.tile([128, RB_PER_TS, SW], BF16, name="out_t", tag="out_t")
                t0s = t0_full[:, s * SW:(s + 1) * SW]
                for rb in range(RB_PER_TS):
                    emit_op(ot[:, rb, :], t0s, g * RB_PER_TS + rb, engs[rb])
                dram = out_d[
                    g * SW:(g + 1) * SW, s * SW:(s + 1) * SW
                ].rearrange("(b p) c -> p b c", b=RB_PER_TS)
                nc.sync.dma_start(dram, ot[:])

            def emit_rows(g, engs="VVVV"):
                """Full-width rows for sq-group g (all of t0 ready): two
                [128, 2, N] units, each 1 MB with 4 KB descriptors."""
                for half in range(2):
                    orow = orow_pool.tile([128, 2, N], BF16, name="out_row", tag="out_r")
                    for rb in range(2):
                        i = g * RB_PER_TS + 2 * half + rb
                        emit_op(orow[:, rb, :], t0_full[:], i, engs[2 * half + rb])
                    i0 = (g * RB_PER_TS + 2 * half) * RBW
                    dram = out_d[i0:i0 + 2 * RBW, :].rearrange(
                        "(b p) c -> p b c", b=2
                    )
                    nc.sync.dma_start(dram, orow[:])

            def stage_A_mm(s):
                h1s[s] = (
                    h1pool.tile([128, KC * SW], BF16, tag="h1", name="h1"),
                    mlp_mm(w1ap, lambda k: xap(s, k)),
                )

            def stage_A_elu(s):
                h, pss = h1s[s]
                mlp_elu(h, pss, b1e, b1r, PLAN[s])
                h1s[s] = h

            def stage_B_mm(s):
                h1 = h1s.pop(s)[:]
                h2s[s] = (
                    h2pool.tile([128, KC * SW], BF16, tag="h2", name="h2"),
                    mlp_mm(w2ap, lambda k: h1[:, k * SW:(k + 1) * SW]),
                )

            def stage_B_elu(s):
                h, pss = h2s[s]
                mlp_elu(h, pss, b2e, b2r, PLAN[s])
                h2s[s] = h

            def compute_scores(s, h2, t0_first):
                """sq + t0 for slice s. Accumulation groups must be
                contiguous (the PE has one open group at a time), so each
                group runs start..stop with k inner; t0_first orders the
                t0 group and copy ahead of sq for the last slice."""
                qps = sqps_pool.tile([128, 2 * RB_PER_TS], F32, name="qps")
                pst = t0ps_pool.tile([128, SW], F32, name="ps_t0")

                def mm_t0():
                    for k in range(KC):
                        nc.tensor.matmul(
                            pst[:], vkbap(k), h2[:, k * SW:(k + 1) * SW],
                            start=(k == 0), stop=(k == KC - 1),
                        )

                def mm_sq():
                    for rb in range(RB_PER_TS):
                        for k in range(KC):
                            nc.tensor.matmul(
                                qps[:, 2 * rb:2 * rb + 2],
                                h2[:, k * SW + rb * RBW:k * SW + (rb + 1) * RBW],
                                vqap[:, 2 * k:2 * k + 2],
                                start=(k == 0),
                                stop=(k == KC - 1),
                            )

                def cp_t0():
                    nc.scalar.activation(
                        t0_full[:, s * SW:(s + 1) * SW], pst[:], AF.Identity,
                        bias=cb,
                    )

                def cp_sq():
                    st = sqpool.tile(
                        [128, 2 * RB_PER_TS], F32, tag=f"sq_{s}", name=f"sq_{s}"
                    )
                    nc.scalar.activation(st[:], qps[:], AF.Identity)
                    sqt[s] = st

                if t0_first:
                    mm_t0(); mm_sq(); cp_t0(); cp_sq()
                else:
                    mm_sq(); mm_t0(); cp_sq(); cp_t0()

            def stage_C(s, engs):
                """t0 slice s (sk'[j] + c, bf16), sq for slice s's row
                blocks, then emit newly-ready units."""
                h2 = h2s.pop(s)[:]
                if s < TS - 1:
                    # sq copy first: it gates every unit of this slice's
                    # rows, t0 only gates this column slice
                    compute_scores(s, h2, t0_first=False)
                    for g in range(s):
                        emit_unit(g, s, engs)
                    if s == 0:
                        emit_unit(0, 0, engs)
                    else:
                        # this slice's new rows across ALL ready t0 columns
                        # in one wide op per row block (two [128, 2, cols]
                        # units): fewer DVE fixed overheads than s+1
                        # separate 512-col units, and 2-3 KB descriptors.
                        cols = (s + 1) * SW
                        for half in range(2):
                            orow = orow_pool.tile(
                                [128, 2, N], BF16, name="out_row", tag="out_r"
                            )
                            for rb in range(2):
                                i = s * RB_PER_TS + 2 * half + rb
                                emit_op(orow[:, rb, 0:cols],
                                        t0_full[:, 0:cols], i,
                                        engs[2 * half + rb])
                            i0 = (s * RB_PER_TS + 2 * half) * RBW
                            dram = out_d[i0:i0 + 2 * RBW, 0:cols].rearrange(
                                "(b p) c -> p b c", b=2
                            )
                            nc.sync.dma_start(dram, orow[:, :, 0:cols])
                else:
                    # last slice: t0 copy first so the old rows' final
                    # column ships while sq(3) is still in flight; then the
                    # new rows go out full-width.
                    compute_scores(s, h2, t0_first=True)
                    for g in range(s):
                        emit_unit(g, s, "SSVS")
                    emit_rows(s, "VVVV")

            # software pipeline: A=L1 matmuls, Ae=L1 ELU, B=L2, C=score+
            # emit. Matmul phases are hoisted so the in-order PE queue is
            # always fed; the hoisted A(s+1) ELU runs right after B(s)'s
            # ELU (before C(s)): that costs C(s)'s first emission ~1.4us
            # of ACT queue, but lets B(s+1) run under C(s)'s emissions -
            # deferring it serialized the whole next-slice chain instead.
            stage_A_mm(0)
            stage_A_elu(0)
            stage_A_mm(1)
            stage_B_mm(0)
            stage_B_elu(0)
            stage_A_elu(1)
            stage_A_mm(2)
            stage_C(0, "VVVV")
            stage_B_mm(1)
            stage_B_elu(1)
            stage_A_elu(2)
            stage_A_mm(3)
            stage_C(1, "VVVV")
            stage_B_mm(2)
            stage_B_elu(2)
            stage_A_elu(3)
            stage_C(2, "VVVV")
            stage_B_mm(3)
            stage_B_elu(3)
            stage_C(3, "VSVS")

    _split_multiwaits(nc)
    return nc


_prog_cache = {}


def _get_program():
    if "nc" not in _prog_cache:
        _prog_cache["nc"] = _build_program()
    return _prog_cache["nc"]


def kernel(**inputs):
    inp = np.asarray(inputs["inputs"], np.float32)        # [B, N, F]
    fc1_w = np.asarray(inputs["fc1_w"], np.float64)
    fc1_b = np.asarray(inputs["fc1_b"], np.float64)
    fc2_w = np.asarray(inputs["fc2_w"], np.float64)
    fc2_b = np.asarray(inputs["fc2_b"], np.float64)
    bn_g = np.asarray(inputs["bn_g"], np.float64)
    bn_b = np.asarray(inputs["bn_b"], np.float64)
    bn_mean = np.asarray(inputs["bn_mean"], np.float64)
    bn_var = np.asarray(inputs["bn_var"], np.float64)
    wq_w = np.asarray(inputs["wq_w"], np.float64)
    wq_b = np.asarray(inputs["wq_b"], np.float64)
    wk_w = np.asarray(inputs["wk_w"], np.float64)
    wk_b = np.asarray(inputs["wk_b"], np.float64)
    mlp2_w = np.asarray(inputs["mlp2_w"], np.float64)
    mlp2_b = np.asarray(inputs["mlp2_b"], np.float64)

    # Fold BN (eval) into the Q/K projections, then both projections and the
    # mlp2 halves into two R^F vectors + one scalar (exact linear algebra).
    D = wq_w.shape[0]
    s = bn_g / np.sqrt(bn_var + 1e-5)
    t = bn_b - bn_mean * s
    wqf = wq_w * s[None, :]
    bqf = wq_b + wq_w @ t
    wkf = wk_w * s[None, :]
    bkf = wk_b + wk_w @ t
    wk_half, wq_half = mlp2_w[0, :D], mlp2_w[0, D:]
    vq = wqf.T @ wq_half
    vk = wkf.T @ wk_half
    c_total = float(bqf @ wq_half + bkf @ wk_half + mlp2_b[0])

    # The device computes h' = ELU(z)+1 per layer; fold the -1 into the
    # consumer: z2 = w2 @ (h1'-1) + b2 = w2 @ h1' + (b2 - w2 @ 1), and
    # sq + sk = vq.h2' + vk.h2' - sum(vq) - sum(vk) (into the c scalar).
    b2f = fc2_b - fc2_w.sum(axis=1)
    c_total = c_total - float(vq.sum() + vk.sum())

    bf = ml_dtypes.bfloat16
    wblob1 = np.zeros((128, W1_COLS), dtype=bf)
    wblob2 = np.zeros((128, WB2_COLS), dtype=bf)
    w1t = fc1_w.T  # [in, out]
    w2t = fc2_w.T
    for k in range(KC):
        wblob1[:, k * F:(k + 1) * F] = w1t[k * 128:(k + 1) * 128, :].astype(bf)
        wblob2[:, W2_OFF + k * F:W2_OFF + (k + 1) * F] = w2t[k * 128:(k + 1) * 128, :].astype(bf)
        wblob2[:, VKB_OFF + k * 128:VKB_OFF + (k + 1) * 128] = np.tile(
            vk[k * 128:(k + 1) * 128, None].astype(bf), (1, 128)
        )
        wblob2[:, VQ_OFF + 2 * k] = vq[k * 128:(k + 1) * 128].astype(bf)

    cst = np.zeros((128, 4 * KC + 1), dtype=np.float32)
    cst[:, 0:KC] = fc1_b.reshape(KC, 128).T
    cst[:, KC:2 * KC] = (fc1_b + 1.0).reshape(KC, 128).T
    cst[:, 2 * KC:3 * KC] = b2f.reshape(KC, 128).T
    cst[:, 3 * KC:4 * KC] = (b2f + 1.0).reshape(KC, 128).T
    cst[:, 4 * KC] = c_total

    shared = {"wb1": wblob1, "wb2": wblob2, "cst": cst}
    in_maps = [
        {"xt": np.ascontiguousarray(inp[b].T).astype(bf), **shared}
        for b in range(B)
    ]

    nc = _get_program()
    res = run_bass_kernel_spmd(nc, in_maps, core_ids=list(range(B)))
    kernel.last_results = res
    return np.stack(
        [np.asarray(res.results[b]["out"]).astype(np.float32) for b in range(B)],
        axis=0,
    )


# revision 43
# speedup vs baseline: 1.0369x; 1.0369x over previous
"""Trainium2 Bass kernel for nn_Encoder_Spatio (gnn_message_passing).

Math (exact reformulation of the reference):
    h1 = ELU(x @ fc1_w.T + b1)
    h2 = ELU(h1 @ fc2_w.T + b2)
    probs[b,i,j] = sq[b,i] + sk[b,j] + c
where BN (eval) and the Q/K projections + mlp2 halves fold on the host into
    vq, vk in R^256,  c scalar:    sq = h2 @ vq,  sk = h2 @ vk  (+consts).

ELU identity used on device:
    ELU(z)+1 = min(exp(z+b), max(z+b+1, 1))
with the -1 folded into the next consumer's bias on the host. Two engine
splits of that identity are used ("plan" per token slice):
    A (latency, slice 0):   e  = Exp(z+b)            ACT, PSUM -> bf16
                            r  = max(z+(b+1), 1)     DVE TS, PSUM -> bf16 (1x)
                            h  = min(e, r)           DVE TT bf16 (2x)
    D (throughput, 1..3):   e  = Exp(z+b)            ACT, PSUM -> bf16
                            u  = z+(b+1)             ACT Identity, PSUM -> bf16
                            r  = max(u, 1)           DVE TS bf16 all-SBUF (4x)
                            h  = min(r, e)           DVE TT bf16 (2x)
Plan D moves the second PSUM read (inherently 1x) onto ACT so DVE keeps only
cheap 2x/4x bf16 ops; the global DVE/ACT load then balances once DVE also
absorbs nearly all score emissions.

The output is written to HBM in bf16 and upcast to fp32 on the host: the
correctness gate is 2e-2 relative error, the bf16 compute path measures
~5e-3, and bf16 rounding of the scores adds ~4e-3 in quadrature. This halves
the dominant HBM write stream (16.8 MB -> 8.4 MB per core) and makes the
emission adds eligible for the DVE 4x perf mode (all-SBUF, 2-byte packed).

Score emission: out[i,j] = t0[j] + sq[i] with t0 = sk + c kept in bf16.
Units of [128, 4 row blocks, 512 cols] (0.5 MB, one tensor_scalar per row
block, ~every 8th op on ACT to balance), plus the last slice's rows as
full-width [128, 2, 2048] units (4 KB descriptors). All output DMAs issue
from the otherwise-idle sync ring.

Sharding: pure data-parallel over B (8 batches -> 8 cores), no collectives.
"""

import sys

if "/opt/trn_rl_repo" not in sys.path:
    sys.path.insert(0, "/opt/trn_rl_repo")

import types

import numpy as np


def _ensure_axon_hooks():
    """concourse.bass_utils imports antenv.axon_hooks when tracing is
    requested; this image's antenv package lacks that submodule, which turns
    a skipped-trace fallback into a hard ImportError. Fill the hole with a
    None-hook stub (tracing degrades gracefully) if it's missing."""
    try:
        import antenv.axon_hooks  # noqa: F401
        return
    except ImportError:
        pass
    try:
        import antenv
    except ImportError:
        return
    mod = types.ModuleType("antenv.axon_hooks")
    mod._hook = None

    def set_axon_ntff_profile_hook(hook):
        mod._hook = hook

    def get_axon_ntff_profile_hook():
        return mod._hook

    mod.set_axon_ntff_profile_hook = set_axon_ntff_profile_hook
    mod.get_axon_ntff_profile_hook = get_axon_ntff_profile_hook
    sys.modules["antenv.axon_hooks"] = mod
    antenv.axon_hooks = mod


_ensure_axon_hooks()

import ml_dtypes

from concourse import bass, tile, mybir
from concourse.bass_utils import run_bass_kernel_spmd

B, N, F = 8, 2048, 256      # batch, tokens, feature width (NIN == NHID == 256)
KC = F // 128               # feature chunks of 128 partitions
TS = 4                      # token slices
SW = N // TS                # slice width (512 = PSUM bank / max matmul moving dim)
RBW = 128                   # output row-block width (partition dim)
RB_PER_TS = SW // RBW       # row blocks per token slice (= sq group size)

F32 = mybir.dt.float32
BF16 = mybir.dt.bfloat16
AF = mybir.ActivationFunctionType
ALU = mybir.AluOpType

# bf16 weight blobs: wb1 = w1 chunks (needed before the first matmul),
# wb2 = w2 chunks + vk-replicated + vq (needed one stage later)
W1_COLS = KC * F
W2_OFF = 0                      # within wb2
VKB_OFF = W2_OFF + KC * F       # KC chunks of [128, 128] (vk replicated)
VQ_OFF = VKB_OFF + KC * 128     # [128, 2*KC] (vq at even cols, zero odd)
WB2_COLS = VQ_OFF + 2 * KC

WARMUP_N = 10               # dummy matmuls for the PE p-state ramp
WARMUP_MOVING = 192
PLAN = {0: "A", 1: "A", 2: "D", 3: "D"}   # ELU engine split per token slice


def _split_multiwaits(nc):
    """This walrus build lowers at most one sync-wait per instruction on some
    instruction classes (the TileContext exit drain trips it). Hoist extra
    waits onto preceding single-wait drains on the same engine."""
    for f in nc.m.functions:
        for bb in f.blocks:
            insts = list(bb.instructions)
            out = []
            changed = False
            for inst in insts:
                si = inst.sync_info
                if si is not None and si.on_wait and len(si.on_wait) > 1:
                    waits = list(si.on_wait)
                    for k, w in enumerate(waits[:-1]):
                        d = mybir.InstDrain(name=f"{inst.name}-ws{k}")
                        d.engine = inst.engine
                        d.sync_info = mybir.SyncInfo(on_wait=[w], on_update=[])
                        out.append(d)
                    inst.sync_info = mybir.SyncInfo(
                        on_wait=[waits[-1]], on_update=list(si.on_update)
                    )
                    changed = True
                out.append(inst)
            if changed:
                bb.instructions = out


def _build_program():
    nc = bass.Bass(trn_type="TRN2")

    xt_d = nc.dram_tensor("xt", [F, N], BF16, kind="ExternalInput")
    wb1_d = nc.dram_tensor("wb1", [128, W1_COLS], BF16, kind="ExternalInput")
    wb2_d = nc.dram_tensor("wb2", [128, WB2_COLS], BF16, kind="ExternalInput")
    cst_d = nc.dram_tensor("cst", [128, 4 * KC + 1], F32, kind="ExternalInput")
    out_d = nc.dram_tensor("out", [N, N], BF16, kind="ExternalOutput")

    with tile.TileContext(nc) as tc:
        with (
            tc.tile_pool(name="wts", bufs=1) as wpool,
            tc.tile_pool(name="eh", bufs=4) as epool,
            tc.tile_pool(name="uh", bufs=4) as upool,
            tc.tile_pool(name="rh", bufs=4) as rpool,
            tc.tile_pool(name="h1", bufs=2) as h1pool,
            tc.tile_pool(name="h2", bufs=2) as h2pool,
            tc.tile_pool(name="t0", bufs=1) as t0pool,
            tc.tile_pool(name="sqs", bufs=TS) as sqpool,
            tc.tile_pool(name="ob", bufs=10) as opool,
            tc.tile_pool(name="obr", bufs=2) as orow_pool,
            tc.tile_pool(name="psmm", bufs=6, space="PSUM") as pspool,
            tc.tile_pool(name="pst0", bufs=1, space="PSUM") as t0ps_pool,
            tc.tile_pool(name="pssq", bufs=1, space="PSUM") as sqps_pool,
        ):
            # ---- weights / constants (live for the whole kernel) ----
            wb1 = wpool.tile([128, W1_COLS], BF16, tag="wb1", name="wb1")
            wb2 = wpool.tile([128, WB2_COLS], BF16, tag="wb2", name="wb2")
            cst = wpool.tile([128, 4 * KC + 1], F32, tag="cst", name="cst")
            x0 = wpool.tile([128, KC, SW], BF16, tag="x0", name="x0")
            x1 = wpool.tile([128, KC, SW], BF16, tag="x1", name="x1")
            x23 = wpool.tile([128, KC, 2 * SW], BF16, tag="x23", name="x23")

            def w1ap(k, oc):
                return wb1[:, k * F + oc * 128:k * F + oc * 128 + 128]

            def w2ap(k, oc):
                return wb2[:, W2_OFF + k * F + oc * 128:W2_OFF + k * F + oc * 128 + 128]

            def vkbap(k):
                return wb2[:, VKB_OFF + k * 128:VKB_OFF + (k + 1) * 128]

            vqap = wb2[:, VQ_OFF:VQ_OFF + 2 * KC]
            b1e = cst[:, 0:KC]
            b1r = cst[:, KC:2 * KC]
            b2e = cst[:, 2 * KC:3 * KC]
            b2r = cst[:, 3 * KC:4 * KC]
            cb = cst[:, 4 * KC:4 * KC + 1]

            def xap(s, k):
                if s == 0:
                    return x0[:, k, :]
                if s == 1:
                    return x1[:, k, :]
                return x23[:, k, (s - 2) * SW:(s - 1) * SW]

            # input on the scalar HWDGE ring, one DMA per pipeline entry
            # point so A(s) only waits on its own slice's completion sem;
            # weights on the sync ring, wb1 first so the first matmul's
            # inputs land first.
            xt_r = xt_d[:].rearrange("(k p) c -> p k c", k=KC)
            dact = wpool.tile([128, 1], F32, tag="dact", name="dact")
            nc.gpsimd.memset(dact[:], 0.0)
            nc.scalar.dma_start(x0[:], xt_r[:, :, 0:SW])
            # dummy Exp right after the x0 trigger: its function-table load
            # (~1.3us of ACT) finishes ~2us before the first real Exp needs
            # it, instead of just barely in time.
            nc.scalar.activation(dact[:], dact[:], AF.Exp)
            nc.scalar.dma_start(x1[:], xt_r[:, :, SW:2 * SW])
            nc.scalar.dma_start(x23[:], xt_r[:, :, 2 * SW:N])
            nc.sync.dma_start(wb1[:], wb1_d[:])
            nc.sync.dma_start(cst[:], cst_d[:])
            nc.sync.dma_start(wb2[:], wb2_d[:])

            # Warmups during the load window: dummy matmuls spin up the PE
            # p-state clock.
            dmw = wpool.tile([128, 128], BF16, tag="dmw", name="dmw")
            dmx = wpool.tile([128, WARMUP_MOVING], BF16, tag="dmx", name="dmx")
            nc.gpsimd.memset(dmw[:], 0.0)
            nc.gpsimd.memset(dmx[:], 0.0)
            dps = pspool.tile([128, SW], F32, name="dps", tag="psmm")
            for w_i in range(WARMUP_N):
                nc.tensor.matmul(dps[:, 0:WARMUP_MOVING], dmw[:], dmx[:],
                                 start=(w_i == 0), stop=(w_i == WARMUP_N - 1))
            # second dummy Exp after the warmups: keeps the ACT queue's
            # pacing ahead of the first real Exp (measured ~1.4us better
            # than without it).
            nc.scalar.activation(dact[:], dact[:], AF.Exp)

            t0_full = t0pool.tile([128, N], BF16, name="t0_full", tag="t0_full")
            sqt = {}       # slice -> sq tile [128, 2*RB_PER_TS] (even cols)
            h1s = {}       # slice -> [128, KC*SW] bf16 tile
            h2s = {}
            n_emit = 0

            def sq_ap(i):
                """Per-partition sq scalar AP for global row block i."""
                return sqt[i // RB_PER_TS][:, 2 * (i % RB_PER_TS):2 * (i % RB_PER_TS) + 1]

            def mlp_mm(wap, rhs):
                """Matmul phase of one layer: KC PSUM tiles, one per output
                chunk. Issued early so the in-order PE queue stays fed."""
                pss = []
                for oc in range(KC):
                    ps = pspool.tile([128, SW], F32, name="ps_mm", tag="psmm")
                    for k in range(KC):
                        nc.tensor.matmul(
                            ps[:], wap(k, oc), rhs(k),
                            start=(k == 0), stop=(k == KC - 1),
                        )
                    pss.append(ps)
                return pss

            def mlp_elu_chunk(h, ps, be, br, oc, plan):
                """ELU+1 for one output chunk, separate from the matmuls so
                a hoisted stage's ELU never sits ahead of the current
                stage's ELU in the ACT/DVE queues."""
                e = epool.tile([128, SW], BF16, name="e_t")
                nc.scalar.activation(e[:], ps[:], AF.Exp, bias=be[:, oc:oc + 1])
                r = rpool.tile([128, SW], BF16, name="r_t")
                if plan == "A":
                    nc.vector.tensor_scalar(
                        r[:], ps[:], br[:, oc:oc + 1], 1.0, ALU.add, ALU.max
                    )
                else:
                    u = upool.tile([128, SW], BF16, name="u_t")
                    nc.scalar.activation(
                        u[:], ps[:], AF.Identity, bias=br[:, oc:oc + 1]
                    )
                    nc.vector.tensor_scalar(r[:], u[:], 1.0, None, ALU.max)
                nc.vector.tensor_tensor(
                    h[:, oc * SW:(oc + 1) * SW], e[:], r[:], ALU.min
                )

            def mlp_elu(h, pss, be, br, plan):
                for oc in range(KC):
                    mlp_elu_chunk(h, pss[oc], be, br, oc, plan)

            def emit_op(dst, src, i, eng):
                """One row block's score add: dst = src + sq[i] (bf16)."""
                if eng == "S":
                    nc.scalar.activation(dst, src, AF.Identity, bias=sq_ap(i))
                else:
                    nc.vector.tensor_scalar(dst, src, sq_ap(i), None, ALU.add)

            def emit_unit(g, s, engs="VVVV"):
                """Ship sq-group g (row blocks 4g..4g+3) at column slice s:
                four [128, SW] adds into one [128, 4, SW] bf16 tile + one
                0.5 MB DMA from the sync ring."""
                ot = opool.tile([128, RB_PER_TS, SW], BF16, name="out_t", tag="out_t")
                t0s = t0_full[:, s * SW:(s + 1) * SW]
                for rb in range(RB_PER_TS):
                    emit_op(ot[:, rb, :], t0s, g * RB_PER_TS + rb, engs[rb])
                dram = out_d[
                    g * SW:(g + 1) * SW, s * SW:(s + 1) * SW
                ].rearrange("(b p) c -> p b c", b=RB_PER_TS)
                nc.sync.dma_start(dram, ot[:])

            def emit_rows(g, engs="VVVV"):
                """Full-width rows for sq-group g (all of t0 ready): two
                [128, 2, N] units, each 1 MB with 4 KB descriptors."""
                for half in range(2):
                    orow = orow_pool.tile([128, 2, N], BF16, name="out_row", tag="out_r")
                    for rb in range(2):
                        i = g * RB_PER_TS + 2 * half + rb
                        emit_op(orow[:, rb, :], t0_full[:], i, engs[2 * half + rb])
                    i0 = (g * RB_PER_TS + 2 * half) * RBW
                    dram = out_d[i0:i0 + 2 * RBW, :].rearrange(
                        "(b p) c -> p b c", b=2
                    )
                    nc.sync.dma_start(dram, orow[:])

            def stage_A_mm(s):
                h1s[s] = (
                    h1pool.tile([128, KC * SW], BF16, tag="h1", name="h1"),
                    mlp_mm(w1ap, lambda k: xap(s, k)),
                )

            def stage_A_elu(s):
                h, pss = h1s[s]
                mlp_elu(h, pss, b1e, b1r, PLAN[s])
                h1s[s] = h

            def stage_B_mm(s):
                h1 = h1s.pop(s)[:]
                h2s[s] = (
                    h2pool.tile([128, KC * SW], BF16, tag="h2", name="h2"),
                    mlp_mm(w2ap, lambda k: h1[:, k * SW:(k + 1) * SW]),
                )

            def stage_B_elu(s):
                h, pss = h2s[s]
                mlp_elu(h, pss, b2e, b2r, PLAN[s])
                h2s[s] = h

            def compute_scores(s, h2, t0_first):
                """sq + t0 for slice s. Accumulation groups must be
                contiguous (the PE has one open group at a time), so each
                group runs start..stop with k inner; t0_first orders the
                t0 group and copy ahead of sq for the last slice."""
                qps = sqps_pool.tile([128, 2 * RB_PER_TS], F32, name="qps")
                pst = t0ps_pool.tile([128, SW], F32, name="ps_t0")

                def mm_t0():
                    for k in range(KC):
                        nc.tensor.matmul(
                            pst[:], vkbap(k), h2[:, k * SW:(k + 1) * SW],
                            start=(k == 0), stop=(k == KC - 1),
                        )

                def mm_sq():
                    for rb in range(RB_PER_TS):
                        for k in range(KC):
                            nc.tensor.matmul(
                                qps[:, 2 * rb:2 * rb + 2],
                                h2[:, k * SW + rb * RBW:k * SW + (rb + 1) * RBW],
                                vqap[:, 2 * k:2 * k + 2],
                                start=(k == 0),
                                stop=(k == KC - 1),
                            )

                def cp_t0():
                    nc.scalar.activation(
                        t0_full[:, s * SW:(s + 1) * SW], pst[:], AF.Identity,
                        bias=cb,
                    )

                def cp_sq():
                    st = sqpool.tile(
                        [128, 2 * RB_PER_TS], F32, tag=f"sq_{s}", name=f"sq_{s}"
                    )
                    nc.scalar.activation(st[:], qps[:], AF.Identity)
                    sqt[s] = st

                if t0_first:
                    mm_t0(); mm_sq(); cp_t0(); cp_sq()
                else:
                    mm_sq(); mm_t0(); cp_sq(); cp_t0()

            def stage_C(s, engs):
                """t0 slice s (sk'[j] + c, bf16), sq for slice s's row
                blocks, then emit newly-ready units."""
                h2 = h2s.pop(s)[:]
                if s < TS - 1:
                    # sq copy first: it gates every unit of this slice's
                    # rows, t0 only gates this column slice
                    compute_scores(s, h2, t0_first=False)
                    for g in range(s):
                        emit_unit(g, s, engs)
                    for sp in range(s + 1):
                        emit_unit(s, sp, engs)
                else:
                    # last slice: the new rows x OLD columns (0:1536) need
                    # only sq(3), not t0(3) - emit them right after the sq
                    # copy, pulling ~4.4us of wide row ops off the final
                    # tail; then the new-column blocks and the rows' last
                    # 512 columns once t0(3) lands.
                    compute_scores(s, h2, t0_first=False)
                    oldc = s * SW
                    for half in range(2):
                        orow = orow_pool.tile(
                            [128, 2, N], BF16, name="out_row", tag="out_r"
                        )
                        for rb in range(2):
                            i = s * RB_PER_TS + 2 * half + rb
                            emit_op(orow[:, rb, 0:oldc],
                                    t0_full[:, 0:oldc], i, "VV"[rb])
                        i0 = (s * RB_PER_TS + 2 * half) * RBW
                        dram = out_d[i0:i0 + 2 * RBW, 0:oldc].rearrange(
                            "(b p) c -> p b c", b=2
                        )
                        nc.sync.dma_start(dram, orow[:, :, 0:oldc])
                    for g in range(s):
                        emit_unit(g, s, "SSVS")
                    t0n = t0_full[:, oldc:N]
                    for half in range(2):
                        ot = opool.tile(
                            [128, RB_PER_TS, SW], BF16, name="out_t", tag="out_t"
                        )
                        for rb in range(2):
                            i = s * RB_PER_TS + 2 * half + rb
                            emit_op(ot[:, rb, :], t0n, i, "VS"[rb])
                        i0 = (s * RB_PER_TS + 2 * half) * RBW
                        dram = out_d[i0:i0 + 2 * RBW, oldc:N].rearrange(
                            "(b p) c -> p b c", b=2
                        )
                        nc.sync.dma_start(dram, ot[:, 0:2, :])

            # software pipeline: A=L1 matmuls, Ae=L1 ELU, B=L2, C=score+
            # emit. Matmul phases are hoisted so the in-order PE queue is
            # always fed; the hoisted A(s+1) ELU runs right after B(s)'s
            # ELU (before C(s)): that costs C(s)'s first emission ~1.4us
            # of ACT queue, but lets B(s+1) run under C(s)'s emissions -
            # deferring it serialized the whole next-slice chain instead.
            stage_A_mm(0)
            stage_A_elu(0)
            stage_A_mm(1)
            stage_B_mm(0)
            stage_B_elu(0)
            stage_A_elu(1)
            stage_A_mm(2)
            stage_C(0, "VVVV")
            stage_B_mm(1)
            stage_B_elu(1)
            stage_A_elu(2)
            stage_A_mm(3)
            stage_C(1, "VVVV")
            stage_B_mm(2)
            stage_B_elu(2)
            stage_A_elu(3)
            stage_C(2, "VVVV")
            stage_B_mm(3)
            stage_B_elu(3)
            stage_C(3, "VSVS")

    _split_multiwaits(nc)
    return nc


_prog_cache = {}


def _get_program():
    if "nc" not in _prog_cache:
        _prog_cache["nc"] = _build_program()
    return _prog_cache["nc"]


def kernel(**inputs):
    inp = np.asarray(inputs["inputs"], np.float32)        # [B, N, F]
    fc1_w = np.asarray(inputs["fc1_w"], np.float64)
    fc1_b = np.asarray(inputs["fc1_b"], np.float64)
    fc2_w = np.asarray(inputs["fc2_w"], np.float64)
    fc2_b = np.asarray(inputs["fc2_b"], np.float64)
    bn_g = np.asarray(inputs["bn_g"], np.float64)
    bn_b = np.asarray(inputs["bn_b"], np.float64)
    bn_mean = np.asarray(inputs["bn_mean"], np.float64)
    bn_var = np.asarray(inputs["bn_var"], np.float64)
    wq_w = np.asarray(inputs["wq_w"], np.float64)
    wq_b = np.asarray(inputs["wq_b"], np.float64)
    wk_w = np.asarray(inputs["wk_w"], np.float64)
    wk_b = np.asarray(inputs["wk_b"], np.float64)
    mlp2_w = np.asarray(inputs["mlp2_w"], np.float64)
    mlp2_b = np.asarray(inputs["mlp2_b"], np.float64)

    # Fold BN (eval) into the Q/K projections, then both projections and the
    # mlp2 halves into two R^F vectors + one scalar (exact linear algebra).
    D = wq_w.shape[0]
    s = bn_g / np.sqrt(bn_var + 1e-5)
    t = bn_b - bn_mean * s
    wqf = wq_w * s[None, :]
    bqf = wq_b + wq_w @ t
    wkf = wk_w * s[None, :]
    bkf = wk_b + wk_w @ t
    wk_half, wq_half = mlp2_w[0, :D], mlp2_w[0, D:]
    vq = wqf.T @ wq_half
    vk = wkf.T @ wk_half
    c_total = float(bqf @ wq_half + bkf @ wk_half + mlp2_b[0])

    # The device computes h' = ELU(z)+1 per layer; fold the -1 into the
    # consumer: z2 = w2 @ (h1'-1) + b2 = w2 @ h1' + (b2 - w2 @ 1), and
    # sq + sk = vq.h2' + vk.h2' - sum(vq) - sum(vk) (into the c scalar).
    b2f = fc2_b - fc2_w.sum(axis=1)
    c_total = c_total - float(vq.sum() + vk.sum())

    bf = ml_dtypes.bfloat16
    wblob1 = np.zeros((128, W1_COLS), dtype=bf)
    wblob2 = np.zeros((128, WB2_COLS), dtype=bf)
    w1t = fc1_w.T  # [in, out]
    w2t = fc2_w.T
    for k in range(KC):
        wblob1[:, k * F:(k + 1) * F] = w1t[k * 128:(k + 1) * 128, :].astype(bf)
        wblob2[:, W2_OFF + k * F:W2_OFF + (k + 1) * F] = w2t[k * 128:(k + 1) * 128, :].astype(bf)
        wblob2[:, VKB_OFF + k * 128:VKB_OFF + (k + 1) * 128] = np.tile(
            vk[k * 128:(k + 1) * 128, None].astype(bf), (1, 128)
        )
        wblob2[:, VQ_OFF + 2 * k] = vq[k * 128:(k + 1) * 128].astype(bf)

    cst = np.zeros((128, 4 * KC + 1), dtype=np.float32)
    cst[:, 0:KC] = fc1_b.reshape(KC, 128).T
    cst[:, KC:2 * KC] = (fc1_b + 1.0).reshape(KC, 128).T
    cst[:, 2 * KC:3 * KC] = b2f.reshape(KC, 128).T
    cst[:, 3 * KC:4 * KC] = (b2f + 1.0).reshape(KC, 128).T
    cst[:, 4 * KC] = c_total

    shared = {"wb1": wblob1, "wb2": wblob2, "cst": cst}
    in_maps = [
        {"xt": np.ascontiguousarray(inp[b].T).astype(bf), **shared}
        for b in range(B)
    ]

    nc = _get_program()
    res = run_bass_kernel_spmd(nc, in_maps, core_ids=list(range(B)))
    kernel.last_results = res
    return np.stack(
        [np.asarray(res.results[b]["out"]).astype(np.float32) for b in range(B)],
        axis=0,
    )
